# revision 6
# baseline (speedup 1.0000x reference)
"""Trainium2 Bass kernel for nn_AttnBFAN (batched attention w/ focal re-norm).

Data-parallel over the batch dim: 128 batches sharded 16-per-core across 8
NeuronCores. Per batch (Q=128, C=1024, D=1024):
    attn = leaky_relu(context @ query^T, 0.1)          (C, Q)
    attn = attn / (||attn||_2 over q)                  l2norm per (b, c)
    p    = softmax(20 * attn^T, axis=c)                (Q, C)
    t    = (p > mean_c p) * p ; re_attn = t / sum_c t
    wcontext = re_attn @ context                       (Q, D)
returns (query, wcontext, re_attn).

v4: bf16 matmul path + aggressive software pipelining.
 - Host pre-casts context/query to bf16 (halves HBM traffic; rel err vs
   fp32 reference ~6e-3) and pre-transposes query to [d, q] chunks.
 - All PE transposes and bmms run in bf16 at 1.0 cycle/row. The l2-norm
   / softmax / focal chain stays f32 (per-c-column norm errors don't
   cancel in the softmax).
 - bmm2 multiplies the unnormalized focal weights t (bf16) and folds
   the 1/sum_c(t) renorm into the PSUM eviction (per-partition scale).
 - PE stream per batch: [ctx^T chunks for b+1] | ones(b) | bmm1(b+1) |
   t^T(b) | bmm2(b) — bmm1 of the next batch fills the softmax-chain
   gap. The l2/softmax PSUM (S) lives in the bmm2 banks so bmm1(b+1)
   can take a0/a1 right after the Prelu eviction.
 - Loads/stores spread over the three dynamic DMA queues (gpsimd SWDGE,
   SP HWDGE, ACT HWDGE).
"""

import os
import numpy as np
import ml_dtypes

import concourse.bacc as bacc
import concourse.mybir as mybir
import concourse.tile as tile
from concourse.bass_utils import run_bass_kernel_spmd
from concourse.masks import make_identity
from concourse.hw_specs import get_activation_tables

F32 = mybir.dt.float32
F32R = mybir.dt.float32r
BF16 = mybir.dt.bfloat16
AX = mybir.AxisListType
ALU = mybir.AluOpType
ACTF = mybir.ActivationFunctionType

NCORES = 8
NB = 128          # total batches
BPC = NB // NCORES  # batches per core
Q = 128
C = 1024
D = 1024
SMOOTH = 20.0

_CACHE = {}


def _build():
    nc = bacc.Bacc("TRN2", target_bir_lowering=False, debug=False,
                   num_devices=NCORES, name="attn_bfan")
    # query pre-transposed+tiled on host: [b, p(=d%128), jd, q] bf16
    q_in = nc.dram_tensor("query", [BPC, 128, 8, Q], BF16, kind="ExternalInput")
    c_in = nc.dram_tensor("context", [BPC, C, D], BF16, kind="ExternalInput")
    re_out = nc.dram_tensor("re_attn", [BPC, Q, C], F32, kind="ExternalOutput")
    wc_out = nc.dram_tensor("wcontext", [BPC, Q, D], F32, kind="ExternalOutput")

    with tile.TileContext(nc) as tc:
        with (
            tc.tile_pool(name="singles", bufs=1) as singles,
            tc.tile_pool(name="ctxp", bufs=3) as ctxp,
            tc.tile_pool(name="ctxtp", bufs=2) as ctxtp,
            tc.tile_pool(name="qp", bufs=2) as qp,
            tc.tile_pool(name="work", bufs=2) as work,
            tc.tile_pool(name="w1", bufs=1) as w1,
            tc.tile_pool(name="tpool", bufs=2) as tpool,
            tc.tile_pool(name="stat", bufs=2) as stat,
            tc.tile_pool(name="ps_a", bufs=1, space="PSUM") as ps_a,
            tc.tile_pool(name="ps_w", bufs=1, space="PSUM") as ps_w,
            tc.tile_pool(name="ps_tp", bufs=2, space="PSUM") as ps_tp,
        ):
            tab_names = list(get_activation_tables("gen3").keys())
            nc.scalar.add_instruction(mybir.InstLoadActFuncSet(
                name=nc.get_next_instruction_name(),
                act_func_set_id=tab_names.index("natural_log_exp_and_others"),
                ins=[], outs=[]))
            ident = singles.tile([128, 128], F32, tag="ident")
            make_identity(nc, ident[:])
            identb = singles.tile([128, 128], BF16, tag="identb")
            nc.vector.tensor_copy(identb[:], ident[:])
            ones_f = singles.tile([128, 128], F32, tag="ones_f")
            nc.vector.memset(ones_f[:], 1.0)
            ones_r = singles.tile([128, 128], F32R, tag="ones_r")
            nc.vector.tensor_copy(ones_r[:], ones_f[:])
            ln20 = singles.tile([128, 1], F32, tag="ln20")
            nc.vector.memset(ln20[:], float(np.log(SMOOTH)))

            ctx_t = [None] * (BPC + 1)   # plain ctx bf16 [128, 8jc, 1024d]
            ctxT_t = [None] * (BPC + 1)  # ctx^T bf16 [128, 8jd, 1024c]
            qT_t = [None] * (BPC + 1)    # q^T bf16 [128, 8jd, 128q]

            def load_batch(b):
                ctx = ctxp.tile([128, 8, D], BF16, tag="ctx", name="ctx")
                ctx_src = c_in[b].rearrange("(jc p) d -> p jc d", p=128)
                nc.gpsimd.dma_start(out=ctx[:, 0:4, :], in_=ctx_src[:, 0:4, :])
                nc.sync.dma_start(out=ctx[:, 4:8, :], in_=ctx_src[:, 4:8, :])
                ctx_t[b] = ctx
                qT = qp.tile([128, 8, Q], BF16, tag="qT", name="qT")
                nc.gpsimd.dma_start(out=qT[:], in_=q_in[b])
                qT_t[b] = qT

            def transpose_pair(b, jd, copy_eng):
                # PE-transpose ctx chunks jd, jd+1 into one 2-bank PSUM tile,
                # evict with a single 2048-elem bf16 copy on copy_eng.
                ctx = ctx_t[b]
                if ctxT_t[b] is None:
                    ctxT_t[b] = ctxtp.tile([128, 8, C], BF16, tag="ctxT",
                                           name="ctxT")
                ctxT = ctxT_t[b]
                tp = ps_tp.tile([128, 16, 128], BF16, tag="tp", name="tp")
                for k in range(2):
                    for jc in range(8):
                        nc.tensor.transpose(
                            tp[:, k * 8 + jc, :],
                            ctx[:, jc, (jd + k) * 128:(jd + k + 1) * 128],
                            identb[:])
                src = tp[:].rearrange("p a b -> p (a b)")
                dst = ctxT[:, jd:jd + 2, :].rearrange("p a b -> p (a b)")
                if copy_eng == "act":
                    nc.scalar.copy(dst, src)
                else:
                    nc.vector.tensor_copy(dst, src)

            def bmm1(b):
                # attn^T (q, c) accumulated over 8 d-chunks -> a0/a1
                a0 = ps_a.tile([128, 512], F32, tag="a0", name="a0")
                a1 = ps_a.tile([128, 512], F32, tag="a1", name="a1")
                qT = qT_t[b]
                ctxT = ctxT_t[b]
                for jd in range(8):
                    st, sp = jd == 0, jd == 7
                    nc.tensor.matmul(a0[:], qT[:, jd, :], ctxT[:, jd, 0:512],
                                     start=st, stop=sp)
                    nc.tensor.matmul(a1[:], qT[:, jd, :], ctxT[:, jd, 512:1024],
                                     start=st, stop=sp)
                return a0, a1

            # ---- prologue: batch 0 fully staged ----
            load_batch(0)
            for jd in (0, 2, 4, 6):
                transpose_pair(0, jd, "vec" if jd != 0 else "act")
            a_cur = bmm1(0)

            for b in range(BPC):
                a0, a1 = a_cur
                if b + 1 < BPC:
                    load_batch(b + 1)

                # ---- leaky relu eviction (frees a0/a1 for bmm1(b+1)) ----
                attn = work.tile([128, C], F32, tag="attn")
                nc.scalar.activation(attn[:, 0:512], a0[:], ACTF.Prelu,
                                     bias=0.0, scale=1.0, alpha=0.1)
                nc.scalar.activation(attn[:, 512:1024], a1[:], ACTF.Prelu,
                                     bias=0.0, scale=1.0, alpha=0.1)
                # squares on DVE (f32r for the ones-matmul)
                sq = w1.tile([128, C], F32R, tag="w1a")
                nc.vector.tensor_mul(sq[:, 0:512], attn[:, 0:512], attn[:, 0:512])
                nc.vector.tensor_mul(sq[:, 512:1024], attn[:, 512:1024],
                                     attn[:, 512:1024])

                # next batch ctx^T chunks 0-3 (ACT copies the first pair in
                # its pre-Ln gap, DVE the second)
                if b + 1 < BPC:
                    transpose_pair(b + 1, 0, "act")
                    transpose_pair(b + 1, 2, "vec")

                # ---- l2 norm: ones-matmul into the bmm2 banks ----
                w0 = ps_w.tile([128, 512], F32, tag="w0", name="w0")
                w2 = ps_w.tile([128, 512], F32, tag="w2", name="w2")
                nc.tensor.matmul(w0[:], ones_r[:], sq[:, 0:512], start=True, stop=True)
                nc.tensor.matmul(w2[:], ones_r[:], sq[:, 512:1024], start=True, stop=True)

                if b + 1 < BPC:
                    transpose_pair(b + 1, 4, "vec")
                    transpose_pair(b + 1, 6, "vec")

                # 20/sqrt(S) = exp(-0.5*ln(S) + ln 20)
                lnS = w1.tile([128, C], F32, tag="w1b")
                nc.scalar.activation(lnS[:, 0:512], w0[:], ACTF.Ln)
                nc.scalar.activation(lnS[:, 512:1024], w2[:], ACTF.Ln)
                rn20 = w1.tile([128, C], F32, tag="w1c")
                nc.scalar.activation(rn20[:], lnS[:], ACTF.Exp,
                                     bias=ln20[:], scale=-0.5)
                u = w1.tile([128, C], F32, tag="w1a")
                nc.vector.tensor_mul(u[:], attn[:], rn20[:])

                # ---- softmax (no max-sub; |20u| <= 20) with fused row-sum ----
                pu = work.tile([128, C], F32, tag="pu")
                rs = stat.tile([128, 1], F32, tag="rs")
                nc.scalar.activation(pu[:], u[:], ACTF.Exp,
                                     bias=0.0, scale=1.0, accum_out=rs[:])

                # ---- focal: t = (pu > rs/C) * pu (bf16), ts = sum_c t ----
                thr = stat.tile([128, 1], F32, tag="thr")
                nc.scalar.mul(thr[:], rs[:], 1.0 / C)
                t = tpool.tile([128, C], BF16, tag="t")
                ts = stat.tile([128, 1], F32, tag="ts")
                nc.vector.scalar_tensor_tensor(
                    out=t[:], in0=pu[:], scalar=thr[:], in1=pu[:],
                    op0=ALU.is_gt, op1=ALU.mult, accum_out=ts[:])
                rinv = stat.tile([128, 1], F32, tag="rinv")
                nc.vector.reciprocal(rinv[:], ts[:])
                # re_attn = t * (1/ts); DMA on the ACT HWDGE queue
                re = work.tile([128, C], F32, tag="re")
                nc.scalar.activation(re[:], t[:], ACTF.Copy, bias=0.0, scale=rinv[:])
                nc.scalar.dma_start(out=re_out[b], in_=re[:])

                # ---- PE: bmm1(b+1) fills the chain gap ----
                if b + 1 < BPC:
                    a_cur = bmm1(b + 1)

                # ---- t^T (bf16 transposes into a ps_tp slot) ----
                tT = qp.tile([128, 8, Q], BF16, tag="tT")
                tpf = ps_tp.tile([128, 16, 128], BF16, tag="tp", name="tpf")
                for jc in range(8):
                    nc.tensor.transpose(
                        tpf[:, jc, :],
                        t[:, jc * 128:(jc + 1) * 128], identb[:])
                nc.vector.tensor_copy(
                    tT[:].rearrange("p a b -> p (a b)"),
                    tpf[:, 0:8, :].rearrange("p a b -> p (a b)"))

                # ---- bmm2: wc = (t @ ctx) * rinv ----
                ctx = ctx_t[b]
                for jc in range(8):
                    st, sp = jc == 0, jc == 7
                    nc.tensor.matmul(w0[:], tT[:, jc, :], ctx[:, jc, 0:512],
                                     start=st, stop=sp)
                    nc.tensor.matmul(w2[:], tT[:, jc, :], ctx[:, jc, 512:1024],
                                     start=st, stop=sp)
                wc = work.tile([128, D], F32, tag="wc")
                nc.scalar.activation(wc[:, 0:512], w0[:], ACTF.Copy,
                                     bias=0.0, scale=rinv[:])
                nc.scalar.activation(wc[:, 512:1024], w2[:], ACTF.Copy,
                                     bias=0.0, scale=rinv[:])
                nc.sync.dma_start(out=wc_out[b], in_=wc[:])
                ctx_t[b] = None
                ctxT_t[b] = None
                qT_t[b] = None

    nc.compile()
    return nc


def kernel(query: np.ndarray, context: np.ndarray):
    query = np.ascontiguousarray(query, dtype=np.float32)
    context = np.ascontiguousarray(context, dtype=np.float32)
    assert query.shape == (NB, Q, D) and context.shape == (NB, C, D)

    if "nc" not in _CACHE:
        _CACHE["nc"] = _build()
    nc = _CACHE["nc"]

    bf16 = ml_dtypes.bfloat16
    # qT host prep: (B, Q, D) -> [b, p, jd, q] where d = jd*128 + p
    qT = np.ascontiguousarray(
        query.transpose(0, 2, 1).reshape(NB, 8, 128, Q).transpose(0, 2, 1, 3)
    ).astype(bf16)
    ctx_bf = context.astype(bf16)

    in_maps = []
    for k in range(NCORES):
        sl = slice(k * BPC, (k + 1) * BPC)
        in_maps.append({"query": qT[sl], "context": ctx_bf[sl]})

    trace = os.environ.get("KERNEL_TRACE", "0") == "1"
    res = run_bass_kernel_spmd(nc, in_maps, core_ids=list(range(NCORES)),
                               trace=trace)
    _CACHE["last_res"] = res

    re_attn = np.concatenate([r["re_attn"] for r in res.results], axis=0)
    wcontext = np.concatenate([r["wcontext"] for r in res.results], axis=0)
    return query, wcontext, re_attn


# revision 10
# speedup vs baseline: 1.1181x; 1.1181x over previous
"""Trainium2 Bass kernel for nn_AttnBFAN (batched attention w/ focal re-norm).

Data-parallel over the batch dim: 128 batches sharded 16-per-core across 8
NeuronCores. Per batch (Q=128, C=1024, D=1024):
    attn = leaky_relu(context @ query^T, 0.1)          (C, Q)
    attn = attn / (||attn||_2 over q)                  l2norm per (b, c)
    p    = softmax(20 * attn^T, axis=c)                (Q, C)
    t    = (p > mean_c p) * p ; re_attn = t / sum_c t
    wcontext = re_attn @ context                       (Q, D)
returns (query, wcontext, re_attn).

v4: bf16 matmul path + aggressive software pipelining.
 - Host pre-casts context/query to bf16 (halves HBM traffic; rel err vs
   fp32 reference ~6e-3) and pre-transposes query to [d, q] chunks.
 - All PE transposes and bmms run in bf16 at 1.0 cycle/row. The l2-norm
   / softmax / focal chain stays f32 (per-c-column norm errors don't
   cancel in the softmax).
 - bmm2 multiplies the unnormalized focal weights t (bf16) and folds
   the 1/sum_c(t) renorm into the PSUM eviction (per-partition scale).
 - PE stream per batch: [ctx^T chunks for b+1] | ones(b) | bmm1(b+1) |
   t^T(b) | bmm2(b) — bmm1 of the next batch fills the softmax-chain
   gap. The l2/softmax PSUM (S) lives in the bmm2 banks so bmm1(b+1)
   can take a0/a1 right after the Prelu eviction.
 - Loads/stores spread over the three dynamic DMA queues (gpsimd SWDGE,
   SP HWDGE, ACT HWDGE).
"""

import os
import numpy as np
import ml_dtypes

import concourse.bacc as bacc
import concourse.mybir as mybir
import concourse.tile as tile
from concourse.bass_utils import run_bass_kernel_spmd
from concourse.masks import make_identity
from concourse.hw_specs import get_activation_tables

F32 = mybir.dt.float32
F32R = mybir.dt.float32r
BF16 = mybir.dt.bfloat16
AX = mybir.AxisListType
ALU = mybir.AluOpType
ACTF = mybir.ActivationFunctionType

NCORES = 8
NB = 128          # total batches
BPC = NB // NCORES  # batches per core
Q = 128
C = 1024
D = 1024
SMOOTH = 20.0

_CACHE = {}


def _build():
    nc = bacc.Bacc("TRN2", target_bir_lowering=False, debug=False,
                   num_devices=NCORES, name="attn_bfan")
    # query pre-transposed+tiled on host: [b, p(=d%128), jd, q] bf16
    q_in = nc.dram_tensor("query", [BPC, 128, 8, Q], BF16, kind="ExternalInput")
    # context pre-tiled on host: [b, p(=c%128), jc, d] bf16 so each
    # partition's DMA line is 16 KB contiguous (descriptor-overhead bound
    # otherwise: 2 KB bf16 rows halve effective DMA rate)
    c_in = nc.dram_tensor("context", [BPC, 128, 8, D], BF16, kind="ExternalInput")
    re_out = nc.dram_tensor("re_attn", [BPC, Q, C], F32, kind="ExternalOutput")
    wc_out = nc.dram_tensor("wcontext", [BPC, Q, D], F32, kind="ExternalOutput")

    with tile.TileContext(nc) as tc:
        with (
            tc.tile_pool(name="singles", bufs=1) as singles,
            tc.tile_pool(name="ctxp", bufs=3) as ctxp,
            tc.tile_pool(name="ctxtp", bufs=2) as ctxtp,
            tc.tile_pool(name="qp", bufs=2) as qp,
            tc.tile_pool(name="work", bufs=2) as work,
            tc.tile_pool(name="w1", bufs=1) as w1,
            tc.tile_pool(name="tpool", bufs=2) as tpool,
            tc.tile_pool(name="stat", bufs=2) as stat,
            tc.tile_pool(name="ps_a", bufs=1, space="PSUM") as ps_a,
            tc.tile_pool(name="ps_w", bufs=1, space="PSUM") as ps_w,
            tc.tile_pool(name="ps_tp", bufs=2, space="PSUM") as ps_tp,
        ):
            tab_names = list(get_activation_tables("gen3").keys())
            nc.scalar.add_instruction(mybir.InstLoadActFuncSet(
                name=nc.get_next_instruction_name(),
                act_func_set_id=tab_names.index("natural_log_exp_and_others"),
                ins=[], outs=[]))
            ident = singles.tile([128, 128], F32, tag="ident")
            make_identity(nc, ident[:])
            identb = singles.tile([128, 128], BF16, tag="identb")
            nc.vector.tensor_copy(identb[:], ident[:])
            ones_f = singles.tile([128, 128], F32, tag="ones_f")
            nc.vector.memset(ones_f[:], 1.0)
            ones_r = singles.tile([128, 128], F32R, tag="ones_r")
            nc.vector.tensor_copy(ones_r[:], ones_f[:])
            ln20 = singles.tile([128, 1], F32, tag="ln20")
            nc.vector.memset(ln20[:], float(np.log(SMOOTH)))

            ctx_t = [None] * (BPC + 1)   # plain ctx bf16 [128, 8jc, 1024d]
            ctxT_t = [None] * (BPC + 1)  # ctx^T bf16 [128, 8jd, 1024c]
            qT_t = [None] * (BPC + 1)    # q^T bf16 [128, 8jd, 128q]

            def load_batch(b):
                ctx = ctxp.tile([128, 8, D], BF16, tag="ctx", name="ctx")
                nc.gpsimd.dma_start(out=ctx[:], in_=c_in[b])
                ctx_t[b] = ctx
                qT = qp.tile([128, 8, Q], BF16, tag="qT", name="qT")
                nc.gpsimd.dma_start(out=qT[:], in_=q_in[b])
                qT_t[b] = qT

            def transpose_pair(b, jd, copy_eng):
                # PE-transpose ctx chunks jd, jd+1 into one 2-bank PSUM tile,
                # evict with a single 2048-elem bf16 copy on copy_eng.
                ctx = ctx_t[b]
                if ctxT_t[b] is None:
                    ctxT_t[b] = ctxtp.tile([128, 8, C], BF16, tag="ctxT",
                                           name="ctxT")
                ctxT = ctxT_t[b]
                tp = ps_tp.tile([128, 16, 128], BF16, tag="tp", name="tp")
                for k in range(2):
                    for jc in range(8):
                        nc.tensor.transpose(
                            tp[:, k * 8 + jc, :],
                            ctx[:, jc, (jd + k) * 128:(jd + k + 1) * 128],
                            identb[:])
                src = tp[:].rearrange("p a b -> p (a b)")
                dst = ctxT[:, jd:jd + 2, :].rearrange("p a b -> p (a b)")
                if copy_eng == "act":
                    nc.scalar.copy(dst, src)
                else:
                    nc.vector.tensor_copy(dst, src)

            def bmm1(b):
                # attn^T (q, c) accumulated over 8 d-chunks -> a0/a1
                a0 = ps_a.tile([128, 512], F32, tag="a0", name="a0")
                a1 = ps_a.tile([128, 512], F32, tag="a1", name="a1")
                qT = qT_t[b]
                ctxT = ctxT_t[b]
                for jd in range(8):
                    st, sp = jd == 0, jd == 7
                    nc.tensor.matmul(a0[:], qT[:, jd, :], ctxT[:, jd, 0:512],
                                     start=st, stop=sp)
                    nc.tensor.matmul(a1[:], qT[:, jd, :], ctxT[:, jd, 512:1024],
                                     start=st, stop=sp)
                return a0, a1

            # ---- prologue: batch 0 fully staged ----
            load_batch(0)
            for jd in (0, 2, 4, 6):
                transpose_pair(0, jd, "vec" if jd != 0 else "act")
            a_cur = bmm1(0)

            for b in range(BPC):
                a0, a1 = a_cur
                if b + 1 < BPC:
                    load_batch(b + 1)

                # ---- leaky relu eviction (frees a0/a1 for bmm1(b+1)) ----
                attn = work.tile([128, C], F32, tag="attn")
                nc.scalar.activation(attn[:, 0:512], a0[:], ACTF.Prelu,
                                     bias=0.0, scale=1.0, alpha=0.1)
                nc.scalar.activation(attn[:, 512:1024], a1[:], ACTF.Prelu,
                                     bias=0.0, scale=1.0, alpha=0.1)
                # squares on DVE (f32r for the ones-matmul)
                sq = w1.tile([128, C], F32R, tag="w1a")
                nc.vector.tensor_mul(sq[:, 0:512], attn[:, 0:512], attn[:, 0:512])
                nc.vector.tensor_mul(sq[:, 512:1024], attn[:, 512:1024],
                                     attn[:, 512:1024])

                # next batch ctx^T chunks 0-3 (ACT copies the first pair in
                # its pre-Ln gap, DVE the second)
                if b + 1 < BPC:
                    transpose_pair(b + 1, 0, "act")
                    transpose_pair(b + 1, 2, "vec")

                # ---- l2 norm: ones-matmul into the bmm2 banks ----
                w0 = ps_w.tile([128, 512], F32, tag="w0", name="w0")
                w2 = ps_w.tile([128, 512], F32, tag="w2", name="w2")
                nc.tensor.matmul(w0[:], ones_r[:], sq[:, 0:512], start=True, stop=True)
                nc.tensor.matmul(w2[:], ones_r[:], sq[:, 512:1024], start=True, stop=True)

                if b + 1 < BPC:
                    transpose_pair(b + 1, 4, "vec")
                    transpose_pair(b + 1, 6, "vec")

                # 20/sqrt(S) = exp(-0.5*ln(S) + ln 20)
                lnS = w1.tile([128, C], F32, tag="w1b")
                nc.scalar.activation(lnS[:, 0:512], w0[:], ACTF.Ln)
                nc.scalar.activation(lnS[:, 512:1024], w2[:], ACTF.Ln)
                rn20 = w1.tile([128, C], F32, tag="w1c")
                nc.scalar.activation(rn20[:], lnS[:], ACTF.Exp,
                                     bias=ln20[:], scale=-0.5)
                u = w1.tile([128, C], F32, tag="w1a")
                nc.vector.tensor_mul(u[:], attn[:], rn20[:])

                # ---- softmax (no max-sub; |20u| <= 20) with fused row-sum ----
                pu = work.tile([128, C], F32, tag="pu")
                rs = stat.tile([128, 1], F32, tag="rs")
                nc.scalar.activation(pu[:], u[:], ACTF.Exp,
                                     bias=0.0, scale=1.0, accum_out=rs[:])

                # ---- focal: t = (pu > rs/C) * pu (bf16), ts = sum_c t ----
                thr = stat.tile([128, 1], F32, tag="thr")
                nc.scalar.mul(thr[:], rs[:], 1.0 / C)
                t = tpool.tile([128, C], BF16, tag="t")
                ts = stat.tile([128, 1], F32, tag="ts")
                nc.vector.scalar_tensor_tensor(
                    out=t[:], in0=pu[:], scalar=thr[:], in1=pu[:],
                    op0=ALU.is_gt, op1=ALU.mult, accum_out=ts[:])
                rinv = stat.tile([128, 1], F32, tag="rinv")
                nc.vector.reciprocal(rinv[:], ts[:])
                # re_attn = t * (1/ts); DMA on the ACT HWDGE queue
                re = work.tile([128, C], F32, tag="re")
                nc.scalar.activation(re[:], t[:], ACTF.Copy, bias=0.0, scale=rinv[:])
                nc.sync.dma_start(out=re_out[b], in_=re[:])

                # ---- PE: bmm1(b+1) fills the chain gap ----
                if b + 1 < BPC:
                    a_cur = bmm1(b + 1)

                # ---- t^T (bf16 transposes into a ps_tp slot) ----
                tT = qp.tile([128, 8, Q], BF16, tag="tT")
                tpf = ps_tp.tile([128, 16, 128], BF16, tag="tp", name="tpf")
                for jc in range(8):
                    nc.tensor.transpose(
                        tpf[:, jc, :],
                        t[:, jc * 128:(jc + 1) * 128], identb[:])
                nc.vector.tensor_copy(
                    tT[:].rearrange("p a b -> p (a b)"),
                    tpf[:, 0:8, :].rearrange("p a b -> p (a b)"))

                # ---- bmm2: wc = (t @ ctx) * rinv ----
                ctx = ctx_t[b]
                for jc in range(8):
                    st, sp = jc == 0, jc == 7
                    nc.tensor.matmul(w0[:], tT[:, jc, :], ctx[:, jc, 0:512],
                                     start=st, stop=sp)
                    nc.tensor.matmul(w2[:], tT[:, jc, :], ctx[:, jc, 512:1024],
                                     start=st, stop=sp)
                wc = work.tile([128, D], F32, tag="wc")
                nc.scalar.activation(wc[:, 0:512], w0[:], ACTF.Copy,
                                     bias=0.0, scale=rinv[:])
                nc.scalar.activation(wc[:, 512:1024], w2[:], ACTF.Copy,
                                     bias=0.0, scale=rinv[:])
                nc.sync.dma_start(out=wc_out[b], in_=wc[:])
                ctx_t[b] = None
                ctxT_t[b] = None
                qT_t[b] = None

    nc.compile()
    return nc


def kernel(query: np.ndarray, context: np.ndarray):
    query = np.ascontiguousarray(query, dtype=np.float32)
    context = np.ascontiguousarray(context, dtype=np.float32)
    assert query.shape == (NB, Q, D) and context.shape == (NB, C, D)

    if "nc" not in _CACHE:
        _CACHE["nc"] = _build()
    nc = _CACHE["nc"]

    bf16 = ml_dtypes.bfloat16
    # qT host prep: (B, Q, D) -> [b, p, jd, q] where d = jd*128 + p
    qT = np.ascontiguousarray(
        query.transpose(0, 2, 1).reshape(NB, 8, 128, Q).transpose(0, 2, 1, 3)
    ).astype(bf16)
    # context: (B, C, D) -> [b, p, jc, d] with c = jc*128 + p
    ctx_bf = np.ascontiguousarray(
        context.reshape(NB, 8, 128, D).transpose(0, 2, 1, 3)
    ).astype(bf16)

    in_maps = []
    for k in range(NCORES):
        sl = slice(k * BPC, (k + 1) * BPC)
        in_maps.append({"query": qT[sl], "context": ctx_bf[sl]})

    trace = os.environ.get("KERNEL_TRACE", "0") == "1"
    res = run_bass_kernel_spmd(nc, in_maps, core_ids=list(range(NCORES)),
                               trace=trace)
    _CACHE["last_res"] = res

    re_attn = np.concatenate([r["re_attn"] for r in res.results], axis=0)
    wcontext = np.concatenate([r["wcontext"] for r in res.results], axis=0)
    return query, wcontext, re_attn


# revision 14
# speedup vs baseline: 1.1275x; 1.0084x over previous
"""Trainium2 Bass kernel for nn_AttnBFAN (batched attention w/ focal re-norm).

Data-parallel over the batch dim: 128 batches sharded 16-per-core across 8
NeuronCores. Per batch (Q=128, C=1024, D=1024):
    attn = leaky_relu(context @ query^T, 0.1)          (C, Q)
    attn = attn / (||attn||_2 over q)                  l2norm per (b, c)
    p    = softmax(20 * attn^T, axis=c)                (Q, C)
    t    = (p > mean_c p) * p ; re_attn = t / sum_c t
    wcontext = re_attn @ context                       (Q, D)
returns (query, wcontext, re_attn).

v4: bf16 matmul path + aggressive software pipelining.
 - Host pre-casts context/query to bf16 (halves HBM traffic; rel err vs
   fp32 reference ~6e-3) and pre-transposes query to [d, q] chunks.
 - All PE transposes and bmms run in bf16 at 1.0 cycle/row. The l2-norm
   / softmax / focal chain stays f32 (per-c-column norm errors don't
   cancel in the softmax).
 - bmm2 multiplies the unnormalized focal weights t (bf16) and folds
   the 1/sum_c(t) renorm into the PSUM eviction (per-partition scale).
 - PE stream per batch: [ctx^T chunks for b+1] | ones(b) | bmm1(b+1) |
   t^T(b) | bmm2(b) — bmm1 of the next batch fills the softmax-chain
   gap. The l2/softmax PSUM (S) lives in the bmm2 banks so bmm1(b+1)
   can take a0/a1 right after the Prelu eviction.
 - Loads/stores spread over the three dynamic DMA queues (gpsimd SWDGE,
   SP HWDGE, ACT HWDGE).
"""

import os
import numpy as np
import ml_dtypes

import concourse.bacc as bacc
import concourse.mybir as mybir
import concourse.tile as tile
from concourse.bass_utils import run_bass_kernel_spmd
from concourse.masks import make_identity
from concourse.hw_specs import get_activation_tables

F32 = mybir.dt.float32
F32R = mybir.dt.float32r
BF16 = mybir.dt.bfloat16
AX = mybir.AxisListType
ALU = mybir.AluOpType
ACTF = mybir.ActivationFunctionType

NCORES = 8
NB = 128          # total batches
BPC = NB // NCORES  # batches per core
Q = 128
C = 1024
D = 1024
SMOOTH = 20.0

_CACHE = {}


def _build():
    nc = bacc.Bacc("TRN2", target_bir_lowering=False, debug=False,
                   num_devices=NCORES, name="attn_bfan")
    # query pre-transposed+tiled on host: [b, p(=d%128), jd, q] bf16
    q_in = nc.dram_tensor("query", [BPC, 128, 8, Q], BF16, kind="ExternalInput")
    # context pre-tiled on host: [b, p(=c%128), jc, d] bf16 so each
    # partition's DMA line is 16 KB contiguous (descriptor-overhead bound
    # otherwise: 2 KB bf16 rows halve effective DMA rate)
    c_in = nc.dram_tensor("context", [BPC, 128, 8, D], BF16, kind="ExternalInput")
    re_out = nc.dram_tensor("re_attn", [BPC, Q, C], F32, kind="ExternalOutput")
    wc_out = nc.dram_tensor("wcontext", [BPC, Q, D], F32, kind="ExternalOutput")

    with tile.TileContext(nc) as tc:
        with (
            tc.tile_pool(name="singles", bufs=1) as singles,
            tc.tile_pool(name="ctxp", bufs=3) as ctxp,
            tc.tile_pool(name="ctxtp", bufs=2) as ctxtp,
            tc.tile_pool(name="qp", bufs=2) as qp,
            tc.tile_pool(name="work", bufs=2) as work,
            tc.tile_pool(name="w1", bufs=1) as w1,
            tc.tile_pool(name="tpool", bufs=2) as tpool,
            tc.tile_pool(name="stat", bufs=2) as stat,
            tc.tile_pool(name="ps_a", bufs=1, space="PSUM") as ps_a,
            tc.tile_pool(name="ps_w", bufs=1, space="PSUM") as ps_w,
            tc.tile_pool(name="ps_tp", bufs=2, space="PSUM") as ps_tp,
        ):
            tab_names = list(get_activation_tables("gen3").keys())
            nc.scalar.add_instruction(mybir.InstLoadActFuncSet(
                name=nc.get_next_instruction_name(),
                act_func_set_id=tab_names.index("natural_log_exp_and_others"),
                ins=[], outs=[]))
            ident = singles.tile([128, 128], F32, tag="ident")
            make_identity(nc, ident[:])
            identb = singles.tile([128, 128], BF16, tag="identb")
            nc.vector.tensor_copy(identb[:], ident[:])
            ones_f = singles.tile([128, 128], F32, tag="ones_f")
            nc.vector.memset(ones_f[:], 1.0)
            ones_r = singles.tile([128, 128], F32R, tag="ones_r")
            nc.vector.tensor_copy(ones_r[:], ones_f[:])
            ln20 = singles.tile([128, 1], F32, tag="ln20")
            nc.vector.memset(ln20[:], float(np.log(SMOOTH)))
            # exp bias ln(C): pu' = C*exp(20u) so the focal test is pu' > rs'
            # (identical to pu > rs/C) without a separate threshold op
            lnC = singles.tile([128, 1], F32, tag="lnC")
            nc.vector.memset(lnC[:], float(np.log(C)))

            ctx_t = [None] * (BPC + 1)   # plain ctx bf16 [128, 8jc, 1024d]
            ctxT_t = [None] * (BPC + 1)  # ctx^T bf16 [128, 8jd, 1024c]
            qT_t = [None] * (BPC + 1)    # q^T bf16 [128, 8jd, 128q]

            def load_batch(b):
                ctx = ctxp.tile([128, 8, D], BF16, tag="ctx", name="ctx")
                nc.gpsimd.dma_start(out=ctx[:], in_=c_in[b])
                ctx_t[b] = ctx
                qT = qp.tile([128, 8, Q], BF16, tag="qT", name="qT")
                nc.gpsimd.dma_start(out=qT[:], in_=q_in[b])
                qT_t[b] = qT

            def transpose_pair(b, jd, copy_eng):
                # PE-transpose ctx chunks jd, jd+1 into one 2-bank PSUM tile,
                # evict with a single 2048-elem bf16 copy on copy_eng.
                ctx = ctx_t[b]
                if ctxT_t[b] is None:
                    ctxT_t[b] = ctxtp.tile([128, 8, C], BF16, tag="ctxT",
                                           name="ctxT")
                ctxT = ctxT_t[b]
                tp = ps_tp.tile([128, 16, 128], BF16, tag="tp", name="tp")
                for k in range(2):
                    for jc in range(8):
                        nc.tensor.transpose(
                            tp[:, k * 8 + jc, :],
                            ctx[:, jc, (jd + k) * 128:(jd + k + 1) * 128],
                            identb[:])
                src = tp[:].rearrange("p a b -> p (a b)")
                dst = ctxT[:, jd:jd + 2, :].rearrange("p a b -> p (a b)")
                if copy_eng == "act":
                    nc.scalar.copy(dst, src)
                else:
                    nc.vector.tensor_copy(dst, src)

            def bmm1(b):
                # attn^T (q, c) accumulated over 8 d-chunks -> a0/a1
                a0 = ps_a.tile([128, 512], F32, tag="a0", name="a0")
                a1 = ps_a.tile([128, 512], F32, tag="a1", name="a1")
                qT = qT_t[b]
                ctxT = ctxT_t[b]
                for jd in range(8):
                    st, sp = jd == 0, jd == 7
                    nc.tensor.matmul(a0[:], qT[:, jd, :], ctxT[:, jd, 0:512],
                                     start=st, stop=sp)
                    nc.tensor.matmul(a1[:], qT[:, jd, :], ctxT[:, jd, 512:1024],
                                     start=st, stop=sp)
                return a0, a1

            # ---- prologue: batch 0 fully staged, batch 1 loading ----
            load_batch(0)
            load_batch(1)
            for jd in (0, 2, 4, 6):
                transpose_pair(0, jd, "vec" if jd != 0 else "act")
            a_cur = bmm1(0)

            for b in range(BPC):
                a0, a1 = a_cur
                if b + 2 < BPC:
                    load_batch(b + 2)

                # ---- leaky relu eviction (frees a0/a1 for bmm1(b+1)) ----
                attn = work.tile([128, C], F32, tag="attn")
                nc.scalar.activation(attn[:, 0:512], a0[:], ACTF.Prelu,
                                     bias=0.0, scale=1.0, alpha=0.1)
                nc.scalar.activation(attn[:, 512:1024], a1[:], ACTF.Prelu,
                                     bias=0.0, scale=1.0, alpha=0.1)
                # squares on DVE (f32r for the ones-matmul)
                sq = w1.tile([128, C], F32R, tag="w1a")
                nc.vector.tensor_mul(sq[:, 0:512], attn[:, 0:512], attn[:, 0:512])
                nc.vector.tensor_mul(sq[:, 512:1024], attn[:, 512:1024],
                                     attn[:, 512:1024])

                # next batch ctx^T chunks 0-3 (ACT copies the first pair in
                # its pre-Ln gap, DVE the second)
                if b + 1 < BPC:
                    transpose_pair(b + 1, 0, "act")
                    transpose_pair(b + 1, 2, "vec")

                # ---- l2 norm: ones-matmul into the bmm2 banks ----
                w0 = ps_w.tile([128, 512], F32, tag="w0", name="w0")
                w2 = ps_w.tile([128, 512], F32, tag="w2", name="w2")
                nc.tensor.matmul(w0[:], ones_r[:], sq[:, 0:512], start=True, stop=True)
                nc.tensor.matmul(w2[:], ones_r[:], sq[:, 512:1024], start=True, stop=True)

                if b + 1 < BPC:
                    transpose_pair(b + 1, 4, "vec")
                    transpose_pair(b + 1, 6, "vec")

                # 20/sqrt(S) = exp(-0.5*ln(S) + ln 20), half-split so ACT/DVE
                # ping-pong: Ln0 -> rn0 -> u0(DVE) -> pu0 while h1 follows
                lnS = w1.tile([128, C], F32, tag="w1b")
                nc.scalar.activation(lnS[:, 0:512], w0[:], ACTF.Ln)
                nc.scalar.activation(lnS[:, 512:1024], w2[:], ACTF.Ln)
                rn20 = w1.tile([128, C], F32, tag="w1c")
                u = w1.tile([128, C], F32, tag="w1a")
                pu = work.tile([128, C], F32, tag="pu")
                rs0 = stat.tile([128, 1], F32, tag="rs0")
                rs1 = stat.tile([128, 1], F32, tag="rs1")
                h0, h1 = slice(0, 512), slice(512, 1024)
                nc.scalar.activation(rn20[:, h0], lnS[:, h0], ACTF.Exp,
                                     bias=ln20[:], scale=-0.5)
                nc.scalar.activation(rn20[:, h1], lnS[:, h1], ACTF.Exp,
                                     bias=ln20[:], scale=-0.5)
                nc.vector.tensor_mul(u[:, h0], attn[:, h0], rn20[:, h0])
                nc.vector.tensor_mul(u[:, h1], attn[:, h1], rn20[:, h1])
                # pu' = C * exp(20u), rs = sum_c pu' per half
                nc.scalar.activation(pu[:, h0], u[:, h0], ACTF.Exp,
                                     bias=lnC[:], scale=1.0, accum_out=rs0[:])
                nc.scalar.activation(pu[:, h1], u[:, h1], ACTF.Exp,
                                     bias=lnC[:], scale=1.0, accum_out=rs1[:])
                rs = stat.tile([128, 1], F32, tag="rs")
                nc.vector.tensor_add(rs[:], rs0[:], rs1[:])

                # ---- focal: t = (pu' > rs') * pu' (bf16), ts = sum_c t ----
                t = tpool.tile([128, C], BF16, tag="t")
                ts = stat.tile([128, 1], F32, tag="ts")
                nc.vector.scalar_tensor_tensor(
                    out=t[:], in0=pu[:], scalar=rs[:], in1=pu[:],
                    op0=ALU.is_gt, op1=ALU.mult, accum_out=ts[:])
                rinv = stat.tile([128, 1], F32, tag="rinv")
                nc.vector.reciprocal(rinv[:], ts[:])
                # re_attn = t * (1/ts); DMA on the ACT HWDGE queue
                re = work.tile([128, C], F32, tag="re")
                nc.scalar.activation(re[:], t[:], ACTF.Copy, bias=0.0, scale=rinv[:])
                nc.sync.dma_start(out=re_out[b], in_=re[:])

                # ---- PE: bmm1(b+1) fills the chain gap ----
                if b + 1 < BPC:
                    a_cur = bmm1(b + 1)

                # ---- t^T (bf16 transposes into a ps_tp slot) ----
                tT = qp.tile([128, 8, Q], BF16, tag="tT")
                tpf = ps_tp.tile([128, 16, 128], BF16, tag="tp", name="tpf")
                for jc in range(8):
                    nc.tensor.transpose(
                        tpf[:, jc, :],
                        t[:, jc * 128:(jc + 1) * 128], identb[:])
                nc.vector.tensor_copy(
                    tT[:].rearrange("p a b -> p (a b)"),
                    tpf[:, 0:8, :].rearrange("p a b -> p (a b)"))

                # ---- bmm2: wc = (t @ ctx) * rinv ----
                ctx = ctx_t[b]
                for jc in range(8):
                    st, sp = jc == 0, jc == 7
                    nc.tensor.matmul(w0[:], tT[:, jc, :], ctx[:, jc, 0:512],
                                     start=st, stop=sp)
                    nc.tensor.matmul(w2[:], tT[:, jc, :], ctx[:, jc, 512:1024],
                                     start=st, stop=sp)
                wc = work.tile([128, D], F32, tag="wc")
                nc.scalar.activation(wc[:, 0:512], w0[:], ACTF.Copy,
                                     bias=0.0, scale=rinv[:])
                nc.scalar.activation(wc[:, 512:1024], w2[:], ACTF.Copy,
                                     bias=0.0, scale=rinv[:])
                nc.sync.dma_start(out=wc_out[b], in_=wc[:])
                ctx_t[b] = None
                ctxT_t[b] = None
                qT_t[b] = None

    nc.compile()
    return nc


def kernel(query: np.ndarray, context: np.ndarray):
    query = np.ascontiguousarray(query, dtype=np.float32)
    context = np.ascontiguousarray(context, dtype=np.float32)
    assert query.shape == (NB, Q, D) and context.shape == (NB, C, D)

    if "nc" not in _CACHE:
        _CACHE["nc"] = _build()
    nc = _CACHE["nc"]

    bf16 = ml_dtypes.bfloat16
    # qT host prep: (B, Q, D) -> [b, p, jd, q] where d = jd*128 + p
    qT = np.ascontiguousarray(
        query.transpose(0, 2, 1).reshape(NB, 8, 128, Q).transpose(0, 2, 1, 3)
    ).astype(bf16)
    # context: (B, C, D) -> [b, p, jc, d] with c = jc*128 + p
    ctx_bf = np.ascontiguousarray(
        context.reshape(NB, 8, 128, D).transpose(0, 2, 1, 3)
    ).astype(bf16)

    in_maps = []
    for k in range(NCORES):
        sl = slice(k * BPC, (k + 1) * BPC)
        in_maps.append({"query": qT[sl], "context": ctx_bf[sl]})

    trace = os.environ.get("KERNEL_TRACE", "0") == "1"
    res = run_bass_kernel_spmd(nc, in_maps, core_ids=list(range(NCORES)),
                               trace=trace)
    _CACHE["last_res"] = res

    re_attn = np.concatenate([r["re_attn"] for r in res.results], axis=0)
    wcontext = np.concatenate([r["wcontext"] for r in res.results], axis=0)
    return query, wcontext, re_attn
